# revision 24
# baseline (speedup 1.0000x reference)
"""Causal self-attention Trainium2 Bass kernel (8 NeuronCores).

Problem: B=2, T=4096, C=512, H=8 heads, D=64 head dim.
  qkv = x @ w_attn.T + b_attn ; causal softmax attention ; y @ w_proj.T + b_proj

Sharding: 16 (batch, head) units over 8 cores -> each core handles one batch
and two adjacent heads (core = b*4 + hp, heads 2hp and 2hp+1). Weights are
sliced per core on the host; each core computes a [C, T] partial of the
output projection for its batch (heads contribution); host sums the 4
partials per batch and transposes back.

v4: S matmuls for the two heads are issued as row-group pairs (lhsT/rhs at
base partitions 0 and 64) so they execute CONCURRENTLY in the PE array
(measured 216ns per 512-col pair vs 427ns each when sequential - K=64
matmuls never stream above 1.2GHz alone). Both heads' scores live in one
[128,1024] PSUM pair tile, giving 1024-col exp instructions on the ACT
engine. The softmax division avoids the old DRAM-bounce partition
broadcast: the PV ones-column denominator (row 64 of o) is copied to bf16
SBUF, broadcast to 64 partitions with a K=1 ones-matmul at base partition
64, reciprocal'd at base 0 (recip_approx_fast requires base 0), and
multiplied into the PV numerator on the GpSimd engine. QKV biases run on
GpSimd, V-path stays on DVE, exp on ACT. The output projection is emitted
per i-tile (both heads finish together now).
"""

import numpy as np

import concourse.bacc as bacc
import concourse.tile as tile
import concourse.mybir as mybir
from concourse import bass_utils
from concourse.bass import AP

F32 = mybir.dt.float32
BF16 = mybir.dt.bfloat16
I32 = mybir.dt.int32
I16 = mybir.dt.int16
AF = mybir.ActivationFunctionType

# Schraudolph fast-exp constants, bf16 variant (DVE path): exp(s) ~=
# bitcast_bf16(int16(2^7*log2e*s + 2^7*(127 - C))), C centers the relative
# error (~+-3%); the int16 bit pattern IS the bf16 result (one DVE op).
EXP_A = float((1 << 7) * 1.4426950408889634)
EXP_B = float((1 << 7) * (127 - 0.04367744))

B, T, C = 2, 4096, 512
H, D = 8, 64
N_CORES = 8
TQ = 512          # query tile (i-tile)
TJ = 128          # key block (j-block)
NI = T // TQ      # 8 i-tiles
NJ = T // TJ      # 32 j-blocks

MM_DT = BF16


def _emit(nc, tc, ctx):
    xT = nc.dram_tensor("xT", [C, T], MM_DT, kind="ExternalInput").ap()
    wqkvT = nc.dram_tensor("wqkvT", [C, 384], MM_DT, kind="ExternalInput").ap()
    bqkv = nc.dram_tensor("bqkv", [128, 3], F32, kind="ExternalInput").ap()
    wpT = nc.dram_tensor("wpT", [128, C], MM_DT, kind="ExternalInput").ap()
    mask01 = nc.dram_tensor("mask01", [128, 128], MM_DT, kind="ExternalInput").ap()
    ident = nc.dram_tensor("ident", [128, 128], MM_DT, kind="ExternalInput").ap()
    outT = nc.dram_tensor("outT", [C, T], F32, kind="ExternalOutput").ap()

    consts = ctx.enter_context(tc.tile_pool(name="consts", bufs=1))
    big = ctx.enter_context(tc.tile_pool(name="big", bufs=1))
    xt_pool = ctx.enter_context(tc.tile_pool(name="xt", bufs=3))
    vt_pool = ctx.enter_context(tc.tile_pool(name="vt", bufs=2))
    pt_pool = ctx.enter_context(tc.tile_pool(name="pp", bufs=4))
    dr_pool = ctx.enter_context(tc.tile_pool(name="dr", bufs=2))
    rc_pool = ctx.enter_context(tc.tile_pool(name="rc", bufs=2))
    yn_pool = ctx.enter_context(tc.tile_pool(name="yn", bufs=2))
    y1_pool = ctx.enter_context(tc.tile_pool(name="y1", bufs=2))
    os_pool = ctx.enter_context(tc.tile_pool(name="osp", bufs=4))
    # PSUM: 8 banks. sab: 2 x [128,1024] f32 (paired S tiles; also the small
    # bf16 V-transpose tiles in the QKV phase). o: 2 x [65,512] (PV
    # accumulators, row 64 = ones-column denominator). q: 2 x [128,512]
    # (QKV chunks, denominator broadcasts, out-proj).
    ps_sab = ctx.enter_context(tc.tile_pool(name="ps_sab", bufs=2, space="PSUM"))
    ps_o = ctx.enter_context(tc.tile_pool(name="ps_o", bufs=2, space="PSUM"))
    ps_q = ctx.enter_context(tc.tile_pool(name="ps_q", bufs=2, space="PSUM"))

    # --- constants ---
    w_sb = consts.tile([128, 4, 384], MM_DT, name="w_sb")
    nc.sync.dma_start(out=w_sb, in_=wqkvT.rearrange("(c p) m -> p c m", p=128))
    wpm_sb = consts.tile([128, C], MM_DT, name="wpm_sb")
    nc.sync.dma_start(out=wpm_sb, in_=wpT)
    bqkv_sb = consts.tile([128, 3], F32, name="bqkv_sb")
    nc.sync.dma_start(out=bqkv_sb, in_=bqkv)
    mask_sb = consts.tile([128, 128], MM_DT, name="mask_sb")
    nc.sync.dma_start(out=mask_sb, in_=mask01)
    id_sb = consts.tile([128, 128], MM_DT, name="id_sb")
    nc.sync.dma_start(out=id_sb, in_=ident)
    ones_sb = consts.tile([128, 64], MM_DT, name="ones_sb")
    nc.gpsimd.memset(ones_sb, 1.0)

    qT_sb = big.tile([128, T], MM_DT, name="qT_sb")
    kT_sb = big.tile([128, T], MM_DT, name="kT_sb")
    # V in natural layout per 128-key block: [j, 0:64]=v_h0 dims, 64=ones,
    # [65:129]=v_h1 dims, 129=ones. Ones LAST per head so the PV output has
    # dims at rows 0:64 and the denominator at row 64.
    v_all = big.tile([128, NJ, 130], MM_DT, name="v_all")
    nc.gpsimd.memset(v_all[:, :, 64:65], 1.0)
    nc.gpsimd.memset(v_all[:, :, 129:130], 1.0)

    # --- QKV projection (transposed layout) ---
    for t in range(NI):
        t0 = t * TQ
        xc_all = xt_pool.tile([128, 4, TQ], MM_DT, name="xc", tag="xc")
        deng = nc.sync if t % 2 == 0 else nc.gpsimd
        deng.dma_start(out=xc_all,
                       in_=xT.rearrange("(c p) t -> p c t", p=128)[:, :, t0:t0 + TQ])
        xcs = [xc_all[:, c, :] for c in range(4)]
        for m in range(3):  # q, k, v rows of the sliced w_attn
            ps = ps_q.tile([128, TQ], F32, name="qkv_ps", tag="q")
            for c in range(4):
                nc.tensor.matmul(
                    ps,
                    lhsT=w_sb[:, c, m * 128:(m + 1) * 128],
                    rhs=xcs[c],
                    start=(c == 0),
                    stop=(c == 3),
                )
            if m == 0:
                # q scale (1/sqrt(D)) is folded into wqkvT/bqkv on the host
                nc.vector.tensor_scalar_add(qT_sb[:, t0:t0 + TQ], ps, bqkv_sb[:, 0:1])
            elif m == 1:
                nc.vector.tensor_scalar_add(kT_sb[:, t0:t0 + TQ], ps, bqkv_sb[:, 1:2])
            else:
                vt = vt_pool.tile([128, TQ], MM_DT, name="vt", tag="vt")
                nc.vector.tensor_scalar_add(vt, ps, bqkv_sb[:, 2:3])
                for s in range(4):
                    n = t * 4 + s
                    tp = ps_sab.tile([128, 128], MM_DT, name="tp", tag="sab")
                    # bf16 PE transpose (exact move): [vdim, t]^T -> [t, vdim]
                    nc.tensor.transpose(tp, vt[:, s * 128:(s + 1) * 128], id_sb)
                    dst = v_all[:, n, :].rearrange("p (g e) -> p g e", g=2, e=65)[:, :, 0:64]
                    src = tp.rearrange("p (g e) -> p g e", g=2, e=64)
                    nc.vector.tensor_copy(dst, src)

    def emit_proj(i_t, ynm):
        # output projection for i-tile i_t (partial out.T, 2 heads);
        # b_proj is added on the host. Emitted one i-tile late so the PE
        # never waits on the division chain.
        i0 = i_t * TQ
        for mc in range(4):
            po = ps_q.tile([128, TQ], F32, name="po", tag="q")
            nc.tensor.matmul(po, lhsT=wpm_sb[:, mc * 128:(mc + 1) * 128],
                             rhs=ynm, start=True, stop=True)
            ob = os_pool.tile([128, TQ], F32, name="ob", tag="os")
            nc.vector.tensor_copy(ob, po)
            nc.sync.dma_start(out=outT[mc * 128:(mc + 1) * 128, i0:i0 + TQ], in_=ob)

    # --- attention: i-outer, J inner; both heads together per J ---
    prev = None
    for i_t in range(NI):
        i0 = i_t * TQ
        njd = i_t * 4  # first diagonal J-block for this i-tile
        o0 = ps_o.tile([65, TQ], F32, name="o0", tag="o")
        o1 = ps_o.tile([65, TQ], F32, name="o1", tag="o")
        os_ = (o0, o1)
        for J in range(njd + 4):
            j0 = J * TJ
            r = max(0, (J - njd) * TJ)  # leading query cols to skip (diag)
            # S pair: h0 at row group (0,0), h1 at (64,0) -> concurrent
            spair = ps_sab.tile([128, 2 * TQ], F32, name="sp", tag="sab")
            for h in range(2):
                hr = slice(h * 64, (h + 1) * 64)
                nc.tensor.matmul(
                    spair[:, h * TQ + r:(h + 1) * TQ],
                    lhsT=kT_sb[hr, j0:j0 + TJ],
                    rhs=qT_sb[hr, i0 + r:i0 + TQ],
                    start=True, stop=True,
                )
            pt = pt_pool.tile([128, 2 * TQ], MM_DT, name="pt", tag="pt")
            if r == 0 and J % 3 == 1:
                # DVE fast-exp (Schraudolph, bf16): affine-to-int16 whose bit
                # pattern is the bf16 exp; pt aliases the int16 tile.
                nc.vector.tensor_scalar(out=pt.bitcast(I16), in0=spair,
                                        scalar1=EXP_A, scalar2=EXP_B,
                                        op0=mybir.AluOpType.mult,
                                        op1=mybir.AluOpType.add)
            elif r == 0:
                nc.scalar.activation(pt, spair, AF.Exp)
            else:
                sp3 = spair.rearrange("p (b c) -> p b c", b=2, c=TQ)[:, :, r:TQ]
                pt3 = pt.rearrange("p (b c) -> p b c", b=2, c=TQ)[:, :, r:TQ]
                nc.scalar.activation(pt3, sp3, AF.Exp)
            if J >= njd:  # diagonal block: triangular mask on the leading 128
                for h in range(2):
                    c0 = h * TQ + r
                    nc.gpsimd.tensor_mul(pt[:, c0:c0 + TJ], pt[:, c0:c0 + TJ], mask_sb)
            for h in range(2):
                nc.tensor.matmul(
                    os_[h][:, r:TQ],
                    lhsT=v_all[:, J, h * 65:(h + 1) * 65],
                    rhs=pt[:, h * TQ + r:(h + 1) * TQ],
                    start=(J == 0),
                    stop=(J == njd + 3),
                )
        # --- softmax division (no DRAM bounce) ---
        # Drain o to SBUF bf16 right away so the PSUM bank frees for the next
        # i-tile's PV; the rest of the chain runs from SBUF (mul on GpSimd).
        ynm = yn_pool.tile([128, TQ], MM_DT, name="ynm", tag="ynm")
        for h in range(2):
            ocp = dr_pool.tile([65, TQ], MM_DT, name="ocp", tag="dr")
            nc.vector.tensor_copy(ocp, os_[h])
            Dp = ps_q.tile([128, TQ], F32, name="Dp", tag="q")
            # K=1 broadcast matmul at base partition 64 -> denom on rows 0:64
            nc.tensor.matmul(Dp[0:64, :], lhsT=ones_sb[64:65, :],
                             rhs=ocp[64:65, :], start=True, stop=True)
            rc = rc_pool.tile([64, TQ], F32, name="rc", tag="rc")
            nc.vector.reciprocal_approx_fast(out=rc, in_=Dp[0:64, :])
            if h == 0:
                nc.gpsimd.tensor_mul(ynm[0:64, :], ocp[0:64, :], rc)
            else:
                y1 = y1_pool.tile([64, TQ], MM_DT, name="y1", tag="y1")
                nc.gpsimd.tensor_mul(y1, ocp[0:64, :], rc)
                nc.gpsimd.dma_start(out=ynm[64:128, :], in_=y1)
        if prev is not None:
            emit_proj(*prev)
        prev = (i_t, ynm)
    emit_proj(*prev)


_CACHED_NC = None


def _build_program():
    global _CACHED_NC
    if _CACHED_NC is not None:
        return _CACHED_NC
    from contextlib import ExitStack
    nc = bacc.Bacc("TRN2", target_bir_lowering=False, debug=False,
                   num_devices=N_CORES)
    with tile.TileContext(nc) as tc:
        with ExitStack() as ctx:
            _emit(nc, tc, ctx)
    nc.compile()
    _CACHED_NC = nc
    return nc


def _host_inputs(x, w_attn, b_attn, w_proj, b_proj):
    """Build the 8 per-core input maps."""
    import ml_dtypes
    mmnp = ml_dtypes.bfloat16 if MM_DT == BF16 else np.float32
    x = np.asarray(x, dtype=np.float32)
    w_attn = np.asarray(w_attn, dtype=np.float32)
    b_attn = np.asarray(b_attn, dtype=np.float32)
    w_proj = np.asarray(w_proj, dtype=np.float32)
    b_proj = np.asarray(b_proj, dtype=np.float32)

    scale = np.float32(1.0 / np.sqrt(D))
    mask = np.triu(np.ones((128, 128), dtype=np.float32))  # keep jj <= ii
    ident = np.eye(128, dtype=np.float32)

    xT_b = [np.ascontiguousarray(x[b].T).astype(mmnp) for b in range(B)]

    in_maps = []
    for core in range(N_CORES):
        b, hp = divmod(core, 4)
        r0 = 2 * hp * 64  # first row of this core's head-pair slice
        qr = w_attn[r0:r0 + 128] * scale
        kr = w_attn[C + r0:C + r0 + 128]
        vr = w_attn[2 * C + r0:2 * C + r0 + 128]
        wqkvT = np.ascontiguousarray(np.concatenate([qr, kr, vr], axis=0).T)
        bq = b_attn[r0:r0 + 128] * scale
        bk = b_attn[C + r0:C + r0 + 128]
        bv = b_attn[2 * C + r0:2 * C + r0 + 128]
        bqkv = np.ascontiguousarray(np.stack([bq, bk, bv], axis=1))
        wpT = np.ascontiguousarray(w_proj[:, r0:r0 + 128].T)
        in_maps.append({
            "xT": xT_b[b],
            "wqkvT": wqkvT.astype(mmnp),
            "bqkv": bqkv,
            "wpT": wpT.astype(mmnp),
            "mask01": mask.astype(mmnp),
            "ident": ident.astype(mmnp),
        })
    return in_maps


def _gather(results, b_proj):
    out = np.empty((B, T, C), dtype=np.float32)
    for b in range(B):
        acc = results[b * 4]["outT"].astype(np.float32).copy()
        for hp in range(1, 4):
            acc += results[b * 4 + hp]["outT"]
        out[b] = acc.T + np.asarray(b_proj, dtype=np.float32)
    return out


def kernel(x, w_attn, b_attn, w_proj, b_proj, _run_kwargs=None):
    nc = _build_program()
    in_maps = _host_inputs(x, w_attn, b_attn, w_proj, b_proj)
    kw = dict(_run_kwargs or {})
    res = bass_utils.run_bass_kernel_spmd(nc, in_maps,
                                          core_ids=list(range(N_CORES)), **kw)
    out = _gather(res.results, b_proj)
    if _run_kwargs is not None:
        kernel.last_result = res
    return out


# revision 25
# speedup vs baseline: 1.2370x; 1.2370x over previous
"""Causal self-attention Trainium2 Bass kernel (8 NeuronCores).

Problem: B=2, T=4096, C=512, H=8 heads, D=64 head dim.
  qkv = x @ w_attn.T + b_attn ; causal softmax attention ; y @ w_proj.T + b_proj

Sharding: 16 (batch, head) units over 8 cores -> each core handles one batch
and two adjacent heads (core = b*4 + hp, heads 2hp and 2hp+1). Weights are
sliced per core on the host; each core computes a [C, T] partial of the
output projection for its batch (heads contribution); host sums the 4
partials per batch and transposes back.

v4: S matmuls for the two heads are issued as row-group pairs (lhsT/rhs at
base partitions 0 and 64) so they execute CONCURRENTLY in the PE array
(measured 216ns per 512-col pair vs 427ns each when sequential - K=64
matmuls never stream above 1.2GHz alone). Both heads' scores live in one
[128,1024] PSUM pair tile, giving 1024-col exp instructions on the ACT
engine. The softmax division avoids the old DRAM-bounce partition
broadcast: the PV ones-column denominator (row 64 of o) is copied to bf16
SBUF, broadcast to 64 partitions with a K=1 ones-matmul at base partition
64, reciprocal'd at base 0 (recip_approx_fast requires base 0), and
multiplied into the PV numerator on the GpSimd engine. QKV biases run on
GpSimd, V-path stays on DVE, exp on ACT. The output projection is emitted
per i-tile (both heads finish together now).
"""

import numpy as np

import concourse.bacc as bacc
import concourse.tile as tile
import concourse.mybir as mybir
from concourse import bass_utils
from concourse.bass import AP

F32 = mybir.dt.float32
BF16 = mybir.dt.bfloat16
I32 = mybir.dt.int32
I16 = mybir.dt.int16
AF = mybir.ActivationFunctionType

# Schraudolph fast-exp constants, bf16 variant (DVE path): exp(s) ~=
# bitcast_bf16(int16(2^7*log2e*s + 2^7*(127 - C))), C centers the relative
# error (~+-3%); the int16 bit pattern IS the bf16 result (one DVE op).
EXP_A = float((1 << 7) * 1.4426950408889634)
EXP_B = float((1 << 7) * (127 - 0.04367744))

B, T, C = 2, 4096, 512
H, D = 8, 64
N_CORES = 8
TQ = 512          # query tile (i-tile)
TJ = 128          # key block (j-block)
NI = T // TQ      # 8 i-tiles
NJ = T // TJ      # 32 j-blocks

MM_DT = BF16


def _emit(nc, tc, ctx):
    xT = nc.dram_tensor("xT", [C, T], MM_DT, kind="ExternalInput").ap()
    wqkvT = nc.dram_tensor("wqkvT", [C, 384], MM_DT, kind="ExternalInput").ap()
    bqkv = nc.dram_tensor("bqkv", [128, 3], F32, kind="ExternalInput").ap()
    wpT = nc.dram_tensor("wpT", [128, C], MM_DT, kind="ExternalInput").ap()
    mask01 = nc.dram_tensor("mask01", [128, 128], MM_DT, kind="ExternalInput").ap()
    ident = nc.dram_tensor("ident", [128, 128], MM_DT, kind="ExternalInput").ap()
    outT = nc.dram_tensor("outT", [C, T], F32, kind="ExternalOutput").ap()

    consts = ctx.enter_context(tc.tile_pool(name="consts", bufs=1))
    big = ctx.enter_context(tc.tile_pool(name="big", bufs=1))
    xt_pool = ctx.enter_context(tc.tile_pool(name="xt", bufs=3))
    vt_pool = ctx.enter_context(tc.tile_pool(name="vt", bufs=2))
    pt_pool = ctx.enter_context(tc.tile_pool(name="pp", bufs=4))
    dr_pool = ctx.enter_context(tc.tile_pool(name="dr", bufs=2))
    rc_pool = ctx.enter_context(tc.tile_pool(name="rc", bufs=2))
    yn_pool = ctx.enter_context(tc.tile_pool(name="yn", bufs=2))
    y1_pool = ctx.enter_context(tc.tile_pool(name="y1", bufs=2))
    os_pool = ctx.enter_context(tc.tile_pool(name="osp", bufs=4))
    # PSUM: 8 banks. sab: 2 x [128,1024] f32 (paired S tiles; also the small
    # bf16 V-transpose tiles in the QKV phase). o: 2 x [65,512] (PV
    # accumulators, row 64 = ones-column denominator). q: 2 x [128,512]
    # (QKV chunks, denominator broadcasts, out-proj).
    ps_sab = ctx.enter_context(tc.tile_pool(name="ps_sab", bufs=2, space="PSUM"))
    ps_o = ctx.enter_context(tc.tile_pool(name="ps_o", bufs=2, space="PSUM"))
    ps_q = ctx.enter_context(tc.tile_pool(name="ps_q", bufs=2, space="PSUM"))

    # --- constants ---
    w_sb = consts.tile([128, 4, 384], MM_DT, name="w_sb")
    nc.sync.dma_start(out=w_sb, in_=wqkvT.rearrange("(c p) m -> p c m", p=128))
    wpm_sb = consts.tile([128, C], MM_DT, name="wpm_sb")
    nc.sync.dma_start(out=wpm_sb, in_=wpT)
    bqkv_sb = consts.tile([128, 3], F32, name="bqkv_sb")
    nc.sync.dma_start(out=bqkv_sb, in_=bqkv)
    mask_sb = consts.tile([128, 128], MM_DT, name="mask_sb")
    nc.sync.dma_start(out=mask_sb, in_=mask01)
    id_sb = consts.tile([128, 128], MM_DT, name="id_sb")
    nc.sync.dma_start(out=id_sb, in_=ident)
    ones_sb = consts.tile([128, 64], MM_DT, name="ones_sb")
    nc.gpsimd.memset(ones_sb, 1.0)

    qT_sb = big.tile([128, T], MM_DT, name="qT_sb")
    kT_sb = big.tile([128, T], MM_DT, name="kT_sb")
    # V in natural layout per 128-key block: [j, 0:64]=v_h0 dims, 64=ones,
    # [65:129]=v_h1 dims, 129=ones. Ones LAST per head so the PV output has
    # dims at rows 0:64 and the denominator at row 64.
    v_all = big.tile([128, NJ, 130], MM_DT, name="v_all")
    nc.gpsimd.memset(v_all[:, :, 64:65], 1.0)
    nc.gpsimd.memset(v_all[:, :, 129:130], 1.0)

    def emit_qkv(t):
        # QKV projection for 512-token tile t (transposed layout); fused into
        # the attention loop right before i-tile t so the PE stays dense.
        t0 = t * TQ
        xc_all = xt_pool.tile([128, 4, TQ], MM_DT, name="xc", tag="xc")
        deng = nc.sync if t % 2 == 0 else nc.gpsimd
        deng.dma_start(out=xc_all,
                       in_=xT.rearrange("(c p) t -> p c t", p=128)[:, :, t0:t0 + TQ])
        xcs = [xc_all[:, c, :] for c in range(4)]
        for m in range(3):  # q, k, v rows of the sliced w_attn
            ps = ps_q.tile([128, TQ], F32, name="qkv_ps", tag="q")
            for c in range(4):
                nc.tensor.matmul(
                    ps,
                    lhsT=w_sb[:, c, m * 128:(m + 1) * 128],
                    rhs=xcs[c],
                    start=(c == 0),
                    stop=(c == 3),
                )
            if m == 0:
                # q scale (1/sqrt(D)) is folded into wqkvT/bqkv on the host
                nc.vector.tensor_scalar_add(qT_sb[:, t0:t0 + TQ], ps, bqkv_sb[:, 0:1])
            elif m == 1:
                nc.vector.tensor_scalar_add(kT_sb[:, t0:t0 + TQ], ps, bqkv_sb[:, 1:2])
            else:
                vt = vt_pool.tile([128, TQ], MM_DT, name="vt", tag="vt")
                nc.vector.tensor_scalar_add(vt, ps, bqkv_sb[:, 2:3])
                for s in range(4):
                    n = t * 4 + s
                    tp = ps_sab.tile([128, 128], MM_DT, name="tp", tag="sab")
                    # bf16 PE transpose (exact move): [vdim, t]^T -> [t, vdim]
                    nc.tensor.transpose(tp, vt[:, s * 128:(s + 1) * 128], id_sb)
                    dst = v_all[:, n, :].rearrange("p (g e) -> p g e", g=2, e=65)[:, :, 0:64]
                    src = tp.rearrange("p (g e) -> p g e", g=2, e=64)
                    nc.vector.tensor_copy(dst, src)

    def emit_proj(i_t, ynm):
        # output projection for i-tile i_t (partial out.T, 2 heads);
        # b_proj is added on the host. Emitted one i-tile late so the PE
        # never waits on the division chain.
        i0 = i_t * TQ
        for mc in range(4):
            po = ps_q.tile([128, TQ], F32, name="po", tag="q")
            nc.tensor.matmul(po, lhsT=wpm_sb[:, mc * 128:(mc + 1) * 128],
                             rhs=ynm, start=True, stop=True)
            ob = os_pool.tile([128, TQ], F32, name="ob", tag="os")
            nc.vector.tensor_copy(ob, po)
            nc.sync.dma_start(out=outT[mc * 128:(mc + 1) * 128, i0:i0 + TQ], in_=ob)

    # --- attention: i-outer, J inner; both heads together per J ---
    prev = None
    for i_t in range(NI):
        emit_qkv(i_t)
        i0 = i_t * TQ
        njd = i_t * 4  # first diagonal J-block for this i-tile
        o0 = ps_o.tile([65, TQ], F32, name="o0", tag="o")
        o1 = ps_o.tile([65, TQ], F32, name="o1", tag="o")
        os_ = (o0, o1)
        for J in range(njd + 4):
            j0 = J * TJ
            r = max(0, (J - njd) * TJ)  # leading query cols to skip (diag)
            # S pair: h0 at row group (0,0), h1 at (64,0) -> concurrent
            spair = ps_sab.tile([128, 2 * TQ], F32, name="sp", tag="sab")
            for h in range(2):
                hr = slice(h * 64, (h + 1) * 64)
                nc.tensor.matmul(
                    spair[:, h * TQ + r:(h + 1) * TQ],
                    lhsT=kT_sb[hr, j0:j0 + TJ],
                    rhs=qT_sb[hr, i0 + r:i0 + TQ],
                    start=True, stop=True,
                )
            pt = pt_pool.tile([128, 2 * TQ], MM_DT, name="pt", tag="pt")
            if r == 0:
                nc.scalar.activation(pt, spair, AF.Exp)
            else:
                sp3 = spair.rearrange("p (b c) -> p b c", b=2, c=TQ)[:, :, r:TQ]
                pt3 = pt.rearrange("p (b c) -> p b c", b=2, c=TQ)[:, :, r:TQ]
                nc.scalar.activation(pt3, sp3, AF.Exp)
            if J >= njd:  # diagonal block: triangular mask on the leading 128
                for h in range(2):
                    c0 = h * TQ + r
                    nc.gpsimd.tensor_mul(pt[:, c0:c0 + TJ], pt[:, c0:c0 + TJ], mask_sb)
            for h in range(2):
                nc.tensor.matmul(
                    os_[h][:, r:TQ],
                    lhsT=v_all[:, J, h * 65:(h + 1) * 65],
                    rhs=pt[:, h * TQ + r:(h + 1) * TQ],
                    start=(J == 0),
                    stop=(J == njd + 3),
                )
        # --- softmax division (no DRAM bounce) ---
        # Drain o to SBUF bf16 right away so the PSUM bank frees for the next
        # i-tile's PV; the rest of the chain runs from SBUF (mul on GpSimd).
        ynm = yn_pool.tile([128, TQ], MM_DT, name="ynm", tag="ynm")
        for h in range(2):
            ocp = dr_pool.tile([65, TQ], MM_DT, name="ocp", tag="dr")
            nc.vector.tensor_copy(ocp, os_[h])
            Dp = ps_q.tile([128, TQ], F32, name="Dp", tag="q")
            # K=1 broadcast matmul at base partition 64 -> denom on rows 0:64
            nc.tensor.matmul(Dp[0:64, :], lhsT=ones_sb[64:65, :],
                             rhs=ocp[64:65, :], start=True, stop=True)
            rc = rc_pool.tile([64, TQ], F32, name="rc", tag="rc")
            nc.vector.reciprocal_approx_fast(out=rc, in_=Dp[0:64, :])
            if h == 0:
                nc.gpsimd.tensor_mul(ynm[0:64, :], ocp[0:64, :], rc)
            else:
                y1 = y1_pool.tile([64, TQ], MM_DT, name="y1", tag="y1")
                nc.gpsimd.tensor_mul(y1, ocp[0:64, :], rc)
                nc.gpsimd.dma_start(out=ynm[64:128, :], in_=y1)
        if prev is not None:
            emit_proj(*prev)
        prev = (i_t, ynm)
    emit_proj(*prev)


_CACHED_NC = None


def _build_program():
    global _CACHED_NC
    if _CACHED_NC is not None:
        return _CACHED_NC
    from contextlib import ExitStack
    nc = bacc.Bacc("TRN2", target_bir_lowering=False, debug=False,
                   num_devices=N_CORES)
    with tile.TileContext(nc) as tc:
        with ExitStack() as ctx:
            _emit(nc, tc, ctx)
    nc.compile()
    _CACHED_NC = nc
    return nc


def _host_inputs(x, w_attn, b_attn, w_proj, b_proj):
    """Build the 8 per-core input maps."""
    import ml_dtypes
    mmnp = ml_dtypes.bfloat16 if MM_DT == BF16 else np.float32
    x = np.asarray(x, dtype=np.float32)
    w_attn = np.asarray(w_attn, dtype=np.float32)
    b_attn = np.asarray(b_attn, dtype=np.float32)
    w_proj = np.asarray(w_proj, dtype=np.float32)
    b_proj = np.asarray(b_proj, dtype=np.float32)

    scale = np.float32(1.0 / np.sqrt(D))
    mask = np.triu(np.ones((128, 128), dtype=np.float32))  # keep jj <= ii
    ident = np.eye(128, dtype=np.float32)

    xT_b = [np.ascontiguousarray(x[b].T).astype(mmnp) for b in range(B)]

    in_maps = []
    for core in range(N_CORES):
        b, hp = divmod(core, 4)
        r0 = 2 * hp * 64  # first row of this core's head-pair slice
        qr = w_attn[r0:r0 + 128] * scale
        kr = w_attn[C + r0:C + r0 + 128]
        vr = w_attn[2 * C + r0:2 * C + r0 + 128]
        wqkvT = np.ascontiguousarray(np.concatenate([qr, kr, vr], axis=0).T)
        bq = b_attn[r0:r0 + 128] * scale
        bk = b_attn[C + r0:C + r0 + 128]
        bv = b_attn[2 * C + r0:2 * C + r0 + 128]
        bqkv = np.ascontiguousarray(np.stack([bq, bk, bv], axis=1))
        wpT = np.ascontiguousarray(w_proj[:, r0:r0 + 128].T)
        in_maps.append({
            "xT": xT_b[b],
            "wqkvT": wqkvT.astype(mmnp),
            "bqkv": bqkv,
            "wpT": wpT.astype(mmnp),
            "mask01": mask.astype(mmnp),
            "ident": ident.astype(mmnp),
        })
    return in_maps


def _gather(results, b_proj):
    out = np.empty((B, T, C), dtype=np.float32)
    for b in range(B):
        acc = results[b * 4]["outT"].astype(np.float32).copy()
        for hp in range(1, 4):
            acc += results[b * 4 + hp]["outT"]
        out[b] = acc.T + np.asarray(b_proj, dtype=np.float32)
    return out


def kernel(x, w_attn, b_attn, w_proj, b_proj, _run_kwargs=None):
    nc = _build_program()
    in_maps = _host_inputs(x, w_attn, b_attn, w_proj, b_proj)
    kw = dict(_run_kwargs or {})
    res = bass_utils.run_bass_kernel_spmd(nc, in_maps,
                                          core_ids=list(range(N_CORES)), **kw)
    out = _gather(res.results, b_proj)
    if _run_kwargs is not None:
        kernel.last_result = res
    return out


# revision 40
# speedup vs baseline: 1.4638x; 1.1834x over previous
"""Causal self-attention Trainium2 Bass kernel (8 NeuronCores).

Problem: B=2, T=4096, C=512, H=8 heads, D=64 head dim.
  qkv = x @ w_attn.T + b_attn ; causal softmax attention ; y @ w_proj.T + b_proj

Sharding: 16 (batch, head) units over 8 cores -> each core handles one batch
and two adjacent heads (core = b*4 + hp, heads 2hp and 2hp+1). Weights are
sliced per core on the host; each core computes a [C, T] partial of the
output projection for its batch; the host sums the 4 partials per batch,
transposes back, and adds b_proj.

Design (measured on HW, see session notes):
- S matmuls for the two heads are issued as row-group pairs (operands at
  base partitions 0 and 64) so they execute CONCURRENTLY in the PE array:
  216ns per 512-col pair vs 427ns each when sequential (K=64 matmuls never
  stream above 1.2GHz alone - half-array activity keeps the HAM clock gate
  cold). Both heads' scores share one [128,1024] PSUM tile.
- exp runs 100%% on the ACT engine ([128,1024] instructions, ~1.0us each,
  back-to-back). Every attempt to offload exp to DVE (Schraudolph bitcast
  tricks, split tiles) LOST time: DVE per-instruction overhead (~350ns) and
  in-order queue coupling with the division chain stall the J-pipeline.
- The causal mask is ADDED (-1e9) into the diagonal S blocks with an
  identity matmul before exp, so no elementwise mask op sits between exp
  and PV. Diagonal S/exp/PV are column-trimmed to the valid range.
- PV keeps M=65 (64 v dims + ones column -> denominator lands in PSUM row
  64). Col-tiled M=64 PV pairs + separate denominator matmuls measured
  SLOWER (extra per-J instructions serialize).
- Softmax division, all off the PE's critical path: o drains to SBUF bf16
  immediately (frees the PSUM bank for the next i-tile), K=1 ones-matmul
  at base partition 64 broadcasts the denominator to rows 0:64,
  reciprocal_approx_fast (requires base partition 0), DVE multiply; the h1
  half is DMA-moved into ynm rows 64:128. The projection output is copied
  f32 and summed on the host (b_proj host-side), QKV biases on DVE.
- QKV stays a separate phase: fusing it into the attention loop couples its
  PSUM pool ring to the division/projection chains and loses 40-75us.
"""

import numpy as np

import concourse.bacc as bacc
import concourse.tile as tile
import concourse.mybir as mybir
from concourse import bass_utils
from concourse.bass import AP

F32 = mybir.dt.float32
BF16 = mybir.dt.bfloat16
I32 = mybir.dt.int32
I16 = mybir.dt.int16
AF = mybir.ActivationFunctionType

# Schraudolph fast-exp constants, bf16 variant (DVE path): exp(s) ~=
# bitcast_bf16(int16(2^7*log2e*s + 2^7*(127 - C))), C centers the relative
# error (~+-3%); the int16 bit pattern IS the bf16 result (one DVE op).
EXP_A = float((1 << 7) * 1.4426950408889634)
EXP_B = float((1 << 7) * (127 - 0.04367744))

B, T, C = 2, 4096, 512
H, D = 8, 64
N_CORES = 8
TQ = 512          # query tile (i-tile)
TJ = 128          # key block (j-block)
NI = T // TQ      # 8 i-tiles
NJ = T // TJ      # 32 j-blocks

MM_DT = BF16


def _emit(nc, tc, ctx):
    xT = nc.dram_tensor("xT", [C, T], MM_DT, kind="ExternalInput").ap()
    wqkvT = nc.dram_tensor("wqkvT", [C, 384], MM_DT, kind="ExternalInput").ap()
    bqkv = nc.dram_tensor("bqkv", [128, 3], F32, kind="ExternalInput").ap()
    wpT = nc.dram_tensor("wpT", [128, C], MM_DT, kind="ExternalInput").ap()
    mask01 = nc.dram_tensor("mask01", [128, 128], MM_DT, kind="ExternalInput").ap()
    ident = nc.dram_tensor("ident", [128, 128], MM_DT, kind="ExternalInput").ap()
    outT = nc.dram_tensor("outT", [C, T], F32, kind="ExternalOutput").ap()

    consts = ctx.enter_context(tc.tile_pool(name="consts", bufs=1))
    big = ctx.enter_context(tc.tile_pool(name="big", bufs=1))
    xt_pool = ctx.enter_context(tc.tile_pool(name="xt", bufs=3))
    vt_pool = ctx.enter_context(tc.tile_pool(name="vt", bufs=2))
    pt_pool = ctx.enter_context(tc.tile_pool(name="pp", bufs=4))
    dr_pool = ctx.enter_context(tc.tile_pool(name="dr", bufs=2))
    rc_pool = ctx.enter_context(tc.tile_pool(name="rc", bufs=2))
    yn_pool = ctx.enter_context(tc.tile_pool(name="yn", bufs=2))
    y1_pool = ctx.enter_context(tc.tile_pool(name="y1", bufs=2))
    os_pool = ctx.enter_context(tc.tile_pool(name="osp", bufs=4))
    # PSUM: 8 banks. sab: 2 x [128,1024] f32 (paired S tiles; also the small
    # bf16 V-transpose tiles in the QKV phase). o: 2 x [65,512] (PV
    # accumulators, row 64 = ones-column denominator). q: 2 x [128,512]
    # (QKV chunks, denominator broadcasts, out-proj).
    ps_sab = ctx.enter_context(tc.tile_pool(name="ps_sab", bufs=2, space="PSUM"))
    ps_o = ctx.enter_context(tc.tile_pool(name="ps_o", bufs=2, space="PSUM"))
    ps_q = ctx.enter_context(tc.tile_pool(name="ps_q", bufs=2, space="PSUM"))

    # --- constants ---
    w_sb = consts.tile([128, 4, 384], MM_DT, name="w_sb")
    nc.sync.dma_start(out=w_sb, in_=wqkvT.rearrange("(c p) m -> p c m", p=128))
    wpm_sb = consts.tile([128, C], MM_DT, name="wpm_sb")
    nc.sync.dma_start(out=wpm_sb, in_=wpT)
    bqkv_sb = consts.tile([128, 3], F32, name="bqkv_sb")
    nc.sync.dma_start(out=bqkv_sb, in_=bqkv)
    mask_sb = consts.tile([128, 128], MM_DT, name="mask_sb")
    nc.sync.dma_start(out=mask_sb, in_=mask01)
    id_sb = consts.tile([128, 128], MM_DT, name="id_sb")
    nc.sync.dma_start(out=id_sb, in_=ident)
    ones_sb = consts.tile([128, 64], MM_DT, name="ones_sb")
    nc.gpsimd.memset(ones_sb, 1.0)

    qT_sb = big.tile([128, T], MM_DT, name="qT_sb")
    kT_sb = big.tile([128, T], MM_DT, name="kT_sb")
    # V in natural layout per 128-key block: [j, 0:64]=v_h0 dims, 64=ones,
    # [65:129]=v_h1 dims, 129=ones. Ones LAST per head so the PV output has
    # dims at rows 0:64 and the denominator at row 64.
    v_all = big.tile([128, NJ, 130], MM_DT, name="v_all")
    nc.gpsimd.memset(v_all[:, :, 64:65], 1.0)
    nc.gpsimd.memset(v_all[:, :, 129:130], 1.0)

    def emit_qkv(t):
        # QKV projection for 512-token tile t (transposed layout); fused into
        # the attention loop right before i-tile t so the PE stays dense.
        t0 = t * TQ
        xc_all = xt_pool.tile([128, 4, TQ], MM_DT, name="xc", tag="xc")
        deng = nc.sync if t % 2 == 0 else nc.gpsimd
        deng.dma_start(out=xc_all,
                       in_=xT.rearrange("(c p) t -> p c t", p=128)[:, :, t0:t0 + TQ])
        xcs = [xc_all[:, c, :] for c in range(4)]
        for m in range(3):  # q, k, v rows of the sliced w_attn
            ps = ps_q.tile([128, TQ], F32, name="qkv_ps", tag="q")
            for c in range(4):
                nc.tensor.matmul(
                    ps,
                    lhsT=w_sb[:, c, m * 128:(m + 1) * 128],
                    rhs=xcs[c],
                    start=(c == 0),
                    stop=(c == 3),
                )
            if m == 0:
                # q scale (1/sqrt(D)) is folded into wqkvT/bqkv on the host
                nc.vector.tensor_scalar_add(qT_sb[:, t0:t0 + TQ], ps, bqkv_sb[:, 0:1])
            elif m == 1:
                nc.vector.tensor_scalar_add(kT_sb[:, t0:t0 + TQ], ps, bqkv_sb[:, 1:2])
            else:
                vt = vt_pool.tile([128, TQ], MM_DT, name="vt", tag="vt")
                nc.vector.tensor_scalar_add(vt, ps, bqkv_sb[:, 2:3])
                for s in range(4):
                    n = t * 4 + s
                    tp = ps_sab.tile([128, 128], MM_DT, name="tp", tag="sab")
                    # bf16 PE transpose (exact move): [vdim, t]^T -> [t, vdim]
                    nc.tensor.transpose(tp, vt[:, s * 128:(s + 1) * 128], id_sb)
                    dst = v_all[:, n, :].rearrange("p (g e) -> p g e", g=2, e=65)[:, :, 0:64]
                    src = tp.rearrange("p (g e) -> p g e", g=2, e=64)
                    nc.vector.tensor_copy(dst, src)

    def emit_proj(i_t, ynm):
        # output projection for i-tile i_t (partial out.T, 2 heads);
        # b_proj is added on the host. Emitted one i-tile late so the PE
        # never waits on the division chain.
        i0 = i_t * TQ
        for mc in range(4):
            po = ps_q.tile([128, TQ], F32, name="po", tag="q")
            nc.tensor.matmul(po, lhsT=wpm_sb[:, mc * 128:(mc + 1) * 128],
                             rhs=ynm, start=True, stop=True)
            ob = os_pool.tile([128, TQ], F32, name="ob", tag="os")
            nc.vector.tensor_copy(ob, po)
            nc.sync.dma_start(out=outT[mc * 128:(mc + 1) * 128, i0:i0 + TQ], in_=ob)

    # --- QKV phase, then attention: i-outer, J inner ---
    for t in range(NI):
        emit_qkv(t)
    for i_t in range(NI):
        i0 = i_t * TQ
        njd = i_t * 4  # first diagonal J-block for this i-tile
        o0 = ps_o.tile([65, TQ], F32, name="o0", tag="o")
        o1 = ps_o.tile([65, TQ], F32, name="o1", tag="o")
        os_ = (o0, o1)
        for J in range(njd + 4):
            j0 = J * TJ
            r = max(0, (J - njd) * TJ)  # leading query cols to skip (diag)
            # S pair: h0 at row group (0,0), h1 at (64,0) -> concurrent
            spair = ps_sab.tile([128, 2 * TQ], F32, name="sp", tag="sab")
            for h in range(2):
                hr = slice(h * 64, (h + 1) * 64)
                nc.tensor.matmul(
                    spair[:, h * TQ + r:(h + 1) * TQ],
                    lhsT=kT_sb[hr, j0:j0 + TJ],
                    rhs=qT_sb[hr, i0 + r:i0 + TQ],
                    start=True, stop=True,
                )
            if J >= njd:
                # diagonal block: accumulate -1e9 onto the invalid (q < k)
                # triangle via an identity matmul, so exp() zeroes it and no
                # separate mask op sits between exp and PV.
                for h in range(2):
                    c0 = h * TQ + r
                    nc.tensor.matmul(spair[:, c0:c0 + TJ], lhsT=id_sb,
                                     rhs=mask_sb, start=False, stop=True,
                                     skip_group_check=True)
            pt = pt_pool.tile([128, 2 * TQ], MM_DT, name="pt", tag="pt")
            if r == 0:
                nc.scalar.activation(pt, spair, AF.Exp)
            else:
                sp3 = spair.rearrange("p (b c) -> p b c", b=2, c=TQ)[:, :, r:TQ]
                pt3 = pt.rearrange("p (b c) -> p b c", b=2, c=TQ)[:, :, r:TQ]
                nc.scalar.activation(pt3, sp3, AF.Exp)
            for h in range(2):
                nc.tensor.matmul(
                    os_[h][:, r:TQ],
                    lhsT=v_all[:, J, h * 65:(h + 1) * 65],
                    rhs=pt[:, h * TQ + r:(h + 1) * TQ],
                    start=(J == 0),
                    stop=(J == njd + 3),
                )
        # --- softmax division (no DRAM bounce) ---
        ynm = yn_pool.tile([128, TQ], MM_DT, name="ynm", tag="ynm")
        for h in range(2):
            ocp = dr_pool.tile([65, TQ], MM_DT, name="ocp", tag="dr")
            nc.vector.tensor_copy(ocp, os_[h])
            Dp = ps_q.tile([128, TQ], F32, name="Dp", tag="q")
            # K=1 broadcast matmul at base partition 64 -> denom on rows 0:64
            nc.tensor.matmul(Dp[0:64, :], lhsT=ones_sb[64:65, :],
                             rhs=ocp[64:65, :], start=True, stop=True)
            rc = rc_pool.tile([64, TQ], F32, name="rc", tag="rc")
            nc.vector.reciprocal_approx_fast(out=rc, in_=Dp[0:64, :])
            if h == 0:
                nc.vector.tensor_mul(ynm[0:64, :], ocp[0:64, :], rc)
            else:
                y1 = y1_pool.tile([64, TQ], MM_DT, name="y1", tag="y1")
                nc.vector.tensor_mul(y1, ocp[0:64, :], rc)
                nc.gpsimd.dma_start(out=ynm[64:128, :], in_=y1)
        emit_proj(i_t, ynm)


_CACHED_NC = None


def _build_program():
    global _CACHED_NC
    if _CACHED_NC is not None:
        return _CACHED_NC
    from contextlib import ExitStack
    nc = bacc.Bacc("TRN2", target_bir_lowering=False, debug=False,
                   num_devices=N_CORES)
    with tile.TileContext(nc) as tc:
        with ExitStack() as ctx:
            _emit(nc, tc, ctx)
    nc.compile()
    _CACHED_NC = nc
    return nc


def _host_inputs(x, w_attn, b_attn, w_proj, b_proj):
    """Build the 8 per-core input maps."""
    import ml_dtypes
    mmnp = ml_dtypes.bfloat16 if MM_DT == BF16 else np.float32
    x = np.asarray(x, dtype=np.float32)
    w_attn = np.asarray(w_attn, dtype=np.float32)
    b_attn = np.asarray(b_attn, dtype=np.float32)
    w_proj = np.asarray(w_proj, dtype=np.float32)
    b_proj = np.asarray(b_proj, dtype=np.float32)

    scale = np.float32(1.0 / np.sqrt(D))
    # additive causal mask: -1e9 where query < key (strict lower triangle)
    mask = np.tril(np.ones((128, 128), dtype=np.float32), -1) * np.float32(-1e9)
    ident = np.eye(128, dtype=np.float32)

    xT_b = [np.ascontiguousarray(x[b].T).astype(mmnp) for b in range(B)]

    in_maps = []
    for core in range(N_CORES):
        b, hp = divmod(core, 4)
        r0 = 2 * hp * 64  # first row of this core's head-pair slice
        qr = w_attn[r0:r0 + 128] * scale
        kr = w_attn[C + r0:C + r0 + 128]
        vr = w_attn[2 * C + r0:2 * C + r0 + 128]
        wqkvT = np.ascontiguousarray(np.concatenate([qr, kr, vr], axis=0).T)
        bq = b_attn[r0:r0 + 128] * scale
        bk = b_attn[C + r0:C + r0 + 128]
        bv = b_attn[2 * C + r0:2 * C + r0 + 128]
        bqkv = np.ascontiguousarray(np.stack([bq, bk, bv], axis=1))
        wpT = np.ascontiguousarray(w_proj[:, r0:r0 + 128].T)
        in_maps.append({
            "xT": xT_b[b],
            "wqkvT": wqkvT.astype(mmnp),
            "bqkv": bqkv,
            "wpT": wpT.astype(mmnp),
            "mask01": mask.astype(mmnp),
            "ident": ident.astype(mmnp),
        })
    return in_maps


def _gather(results, b_proj):
    out = np.empty((B, T, C), dtype=np.float32)
    for b in range(B):
        acc = results[b * 4]["outT"].astype(np.float32).copy()
        for hp in range(1, 4):
            acc += results[b * 4 + hp]["outT"]
        out[b] = acc.T + np.asarray(b_proj, dtype=np.float32)
    return out


def kernel(x, w_attn, b_attn, w_proj, b_proj, _run_kwargs=None):
    nc = _build_program()
    in_maps = _host_inputs(x, w_attn, b_attn, w_proj, b_proj)
    kw = dict(_run_kwargs or {})
    res = bass_utils.run_bass_kernel_spmd(nc, in_maps,
                                          core_ids=list(range(N_CORES)), **kw)
    out = _gather(res.results, b_proj)
    if _run_kwargs is not None:
        kernel.last_result = res
    return out


# revision 42
# speedup vs baseline: 1.4648x; 1.0007x over previous
"""Causal self-attention Trainium2 Bass kernel (8 NeuronCores).

Problem: B=2, T=4096, C=512, H=8 heads, D=64 head dim.
  qkv = x @ w_attn.T + b_attn ; causal softmax attention ; y @ w_proj.T + b_proj

Sharding: 16 (batch, head) units over 8 cores -> each core handles one batch
and two adjacent heads (core = b*4 + hp, heads 2hp and 2hp+1). Weights are
sliced per core on the host; each core computes a [C, T] partial of the
output projection for its batch (heads contribution); host sums the 4
partials per batch and transposes back.

v4: S matmuls for the two heads are issued as row-group pairs (lhsT/rhs at
base partitions 0 and 64) so they execute CONCURRENTLY in the PE array
(measured 216ns per 512-col pair vs 427ns each when sequential - K=64
matmuls never stream above 1.2GHz alone). Both heads' scores live in one
[128,1024] PSUM pair tile, giving 1024-col exp instructions on the ACT
engine. The softmax division avoids the old DRAM-bounce partition
broadcast: the PV ones-column denominator (row 64 of o) is copied to bf16
SBUF, broadcast to 64 partitions with a K=1 ones-matmul at base partition
64, reciprocal'd at base 0 (recip_approx_fast requires base 0), and
multiplied into the PV numerator on the DVE. The o accumulator drains to
SBUF bf16 immediately after the last PV so its PSUM bank frees for the
next i-tile. The causal mask is ADDED (-1e9, identity matmul) into the
diagonal S blocks before exp, so no mask op sits between exp and PV. exp
runs 100% on the ACT engine: every attempt to offload it to DVE/GpSimd
(Schraudolph bitcast, split tiles) or to re-order emission (deferred or
skewed projection, QKV fusion, i-tile interleaving) measured SLOWER on HW
- see versions/ and the auto-memory notes. b_proj is added on the host.
"""

import numpy as np

import concourse.bacc as bacc
import concourse.tile as tile
import concourse.mybir as mybir
from concourse import bass_utils
from concourse.bass import AP

F32 = mybir.dt.float32
BF16 = mybir.dt.bfloat16
I32 = mybir.dt.int32
I16 = mybir.dt.int16
AF = mybir.ActivationFunctionType

# Schraudolph fast-exp constants, bf16 variant (DVE path): exp(s) ~=
# bitcast_bf16(int16(2^7*log2e*s + 2^7*(127 - C))), C centers the relative
# error (~+-3%); the int16 bit pattern IS the bf16 result (one DVE op).
EXP_A = float((1 << 7) * 1.4426950408889634)
EXP_B = float((1 << 7) * (127 - 0.04367744))

B, T, C = 2, 4096, 512
H, D = 8, 64
N_CORES = 8
TQ = 512          # query tile (i-tile)
TJ = 128          # key block (j-block)
NI = T // TQ      # 8 i-tiles
NJ = T // TJ      # 32 j-blocks

MM_DT = BF16


def _emit(nc, tc, ctx):
    xT = nc.dram_tensor("xT", [C, T], MM_DT, kind="ExternalInput").ap()
    wqkvT = nc.dram_tensor("wqkvT", [C, 384], MM_DT, kind="ExternalInput").ap()
    bqkv = nc.dram_tensor("bqkv", [128, 3], F32, kind="ExternalInput").ap()
    wpT = nc.dram_tensor("wpT", [128, C], MM_DT, kind="ExternalInput").ap()
    mask01 = nc.dram_tensor("mask01", [128, 128], MM_DT, kind="ExternalInput").ap()
    ident = nc.dram_tensor("ident", [128, 128], MM_DT, kind="ExternalInput").ap()
    outT = nc.dram_tensor("outT", [C, T], F32, kind="ExternalOutput").ap()

    consts = ctx.enter_context(tc.tile_pool(name="consts", bufs=1))
    big = ctx.enter_context(tc.tile_pool(name="big", bufs=1))
    xt_pool = ctx.enter_context(tc.tile_pool(name="xt", bufs=3))
    vt_pool = ctx.enter_context(tc.tile_pool(name="vt", bufs=2))
    pt_pool = ctx.enter_context(tc.tile_pool(name="pp", bufs=4))
    dr_pool = ctx.enter_context(tc.tile_pool(name="dr", bufs=2))
    rc_pool = ctx.enter_context(tc.tile_pool(name="rc", bufs=2))
    yn_pool = ctx.enter_context(tc.tile_pool(name="yn", bufs=2))
    y1_pool = ctx.enter_context(tc.tile_pool(name="y1", bufs=2))
    os_pool = ctx.enter_context(tc.tile_pool(name="osp", bufs=4))
    # PSUM: 8 banks. sab: 2 x [128,1024] f32 (paired S tiles; also the small
    # bf16 V-transpose tiles in the QKV phase). o: 2 x [65,512] (PV
    # accumulators, row 64 = ones-column denominator). q: 2 x [128,512]
    # (QKV chunks, denominator broadcasts, out-proj).
    ps_sab = ctx.enter_context(tc.tile_pool(name="ps_sab", bufs=2, space="PSUM"))
    ps_o = ctx.enter_context(tc.tile_pool(name="ps_o", bufs=2, space="PSUM"))
    ps_q = ctx.enter_context(tc.tile_pool(name="ps_q", bufs=2, space="PSUM"))

    # --- constants ---
    w_sb = consts.tile([128, 4, 384], MM_DT, name="w_sb")
    nc.sync.dma_start(out=w_sb, in_=wqkvT.rearrange("(c p) m -> p c m", p=128))
    wpm_sb = consts.tile([128, C], MM_DT, name="wpm_sb")
    nc.sync.dma_start(out=wpm_sb, in_=wpT)
    bqkv_sb = consts.tile([128, 3], F32, name="bqkv_sb")
    nc.sync.dma_start(out=bqkv_sb, in_=bqkv)
    mask_sb = consts.tile([128, 128], MM_DT, name="mask_sb")
    nc.sync.dma_start(out=mask_sb, in_=mask01)
    id_sb = consts.tile([128, 128], MM_DT, name="id_sb")
    nc.sync.dma_start(out=id_sb, in_=ident)
    ones_sb = consts.tile([128, 64], MM_DT, name="ones_sb")
    nc.gpsimd.memset(ones_sb, 1.0)

    qT_sb = big.tile([128, T], MM_DT, name="qT_sb")
    kT_sb = big.tile([128, T], MM_DT, name="kT_sb")
    # V in natural layout per 128-key block: [j, 0:64]=v_h0 dims, 64=ones,
    # [65:129]=v_h1 dims, 129=ones. Ones LAST per head so the PV output has
    # dims at rows 0:64 and the denominator at row 64.
    v_all = big.tile([128, NJ, 130], MM_DT, name="v_all")
    nc.gpsimd.memset(v_all[:, :, 64:65], 1.0)
    nc.gpsimd.memset(v_all[:, :, 129:130], 1.0)

    def emit_qkv(t):
        # QKV projection for 512-token tile t (transposed layout); fused into
        # the attention loop right before i-tile t so the PE stays dense.
        t0 = t * TQ
        xc_all = xt_pool.tile([128, 4, TQ], MM_DT, name="xc", tag="xc")
        deng = nc.sync if t % 2 == 0 else nc.gpsimd
        deng.dma_start(out=xc_all,
                       in_=xT.rearrange("(c p) t -> p c t", p=128)[:, :, t0:t0 + TQ])
        xcs = [xc_all[:, c, :] for c in range(4)]
        for m in range(3):  # q, k, v rows of the sliced w_attn
            ps = ps_q.tile([128, TQ], F32, name="qkv_ps", tag="q")
            for c in range(4):
                nc.tensor.matmul(
                    ps,
                    lhsT=w_sb[:, c, m * 128:(m + 1) * 128],
                    rhs=xcs[c],
                    start=(c == 0),
                    stop=(c == 3),
                )
            if m == 0:
                # q scale (1/sqrt(D)) is folded into wqkvT/bqkv on the host
                nc.vector.tensor_scalar_add(qT_sb[:, t0:t0 + TQ], ps, bqkv_sb[:, 0:1])
            elif m == 1:
                nc.vector.tensor_scalar_add(kT_sb[:, t0:t0 + TQ], ps, bqkv_sb[:, 1:2])
            else:
                vt = vt_pool.tile([128, TQ], MM_DT, name="vt", tag="vt")
                nc.vector.tensor_scalar_add(vt, ps, bqkv_sb[:, 2:3])
                for s in range(4):
                    n = t * 4 + s
                    tp = ps_sab.tile([128, 128], MM_DT, name="tp", tag="sab")
                    # bf16 PE transpose (exact move): [vdim, t]^T -> [t, vdim]
                    nc.tensor.transpose(tp, vt[:, s * 128:(s + 1) * 128], id_sb)
                    dst = v_all[:, n, :].rearrange("p (g e) -> p g e", g=2, e=65)[:, :, 0:64]
                    src = tp.rearrange("p (g e) -> p g e", g=2, e=64)
                    nc.vector.tensor_copy(dst, src)

    def emit_proj(i_t, ynm):
        # output projection for i-tile i_t (partial out.T, 2 heads);
        # b_proj is added on the host. Emitted one i-tile late so the PE
        # never waits on the division chain.
        i0 = i_t * TQ
        for mc in range(4):
            po = ps_q.tile([128, TQ], F32, name="po", tag="q")
            nc.tensor.matmul(po, lhsT=wpm_sb[:, mc * 128:(mc + 1) * 128],
                             rhs=ynm, start=True, stop=True)
            ob = os_pool.tile([128, TQ], F32, name="ob", tag="os")
            nc.vector.tensor_copy(ob, po)
            nc.sync.dma_start(out=outT[mc * 128:(mc + 1) * 128, i0:i0 + TQ], in_=ob)

    # --- QKV phase, then attention: i-outer, J inner ---
    for t in range(NI):
        emit_qkv(t)
    for i_t in range(NI):
        i0 = i_t * TQ
        njd = i_t * 4  # first diagonal J-block for this i-tile
        o0 = ps_o.tile([65, TQ], F32, name="o0", tag="o")
        o1 = ps_o.tile([65, TQ], F32, name="o1", tag="o")
        os_ = (o0, o1)
        for J in range(njd + 4):
            j0 = J * TJ
            r = max(0, (J - njd) * TJ)  # leading query cols to skip (diag)
            # S pair: h0 at row group (0,0), h1 at (64,0) -> concurrent
            spair = ps_sab.tile([128, 2 * TQ], F32, name="sp", tag="sab")
            for h in range(2):
                hr = slice(h * 64, (h + 1) * 64)
                nc.tensor.matmul(
                    spair[:, h * TQ + r:(h + 1) * TQ],
                    lhsT=kT_sb[hr, j0:j0 + TJ],
                    rhs=qT_sb[hr, i0 + r:i0 + TQ],
                    start=True, stop=True,
                )
            if J >= njd:
                # diagonal block: accumulate -1e9 onto the invalid (q < k)
                # triangle via an identity matmul, so exp() zeroes it and no
                # separate mask op sits between exp and PV.
                for h in range(2):
                    c0 = h * TQ + r
                    nc.tensor.matmul(spair[:, c0:c0 + TJ], lhsT=id_sb,
                                     rhs=mask_sb, start=False, stop=True,
                                     skip_group_check=True)
            pt = pt_pool.tile([128, 2 * TQ], MM_DT, name="pt", tag="pt")
            if r == 0:
                nc.scalar.activation(pt, spair, AF.Exp)
            else:
                sp3 = spair.rearrange("p (b c) -> p b c", b=2, c=TQ)[:, :, r:TQ]
                pt3 = pt.rearrange("p (b c) -> p b c", b=2, c=TQ)[:, :, r:TQ]
                nc.scalar.activation(pt3, sp3, AF.Exp)
            for h in range(2):
                nc.tensor.matmul(
                    os_[h][:, r:TQ],
                    lhsT=v_all[:, J, h * 65:(h + 1) * 65],
                    rhs=pt[:, h * TQ + r:(h + 1) * TQ],
                    start=(J == 0),
                    stop=(J == njd + 3),
                )
        # --- softmax division (no DRAM bounce) ---
        ynm = yn_pool.tile([128, TQ], MM_DT, name="ynm", tag="ynm")
        for h in range(2):
            ocp = dr_pool.tile([65, TQ], MM_DT, name="ocp", tag="dr")
            nc.vector.tensor_copy(ocp, os_[h])
            Dp = ps_q.tile([128, TQ], F32, name="Dp", tag="q")
            # K=1 broadcast matmul at base partition 64 -> denom on rows 0:64
            nc.tensor.matmul(Dp[0:64, :], lhsT=ones_sb[64:65, :],
                             rhs=ocp[64:65, :], start=True, stop=True)
            rc = rc_pool.tile([64, TQ], F32, name="rc", tag="rc")
            nc.vector.reciprocal_approx_fast(out=rc, in_=Dp[0:64, :])
            if h == 0:
                nc.vector.tensor_mul(ynm[0:64, :], ocp[0:64, :], rc)
            else:
                y1 = y1_pool.tile([64, TQ], MM_DT, name="y1", tag="y1")
                nc.vector.tensor_mul(y1, ocp[0:64, :], rc)
                nc.gpsimd.dma_start(out=ynm[64:128, :], in_=y1)
        emit_proj(i_t, ynm)


_CACHED_NC = None


def _build_program():
    global _CACHED_NC
    if _CACHED_NC is not None:
        return _CACHED_NC
    from contextlib import ExitStack
    nc = bacc.Bacc("TRN2", target_bir_lowering=False, debug=False,
                   num_devices=N_CORES)
    with tile.TileContext(nc) as tc:
        with ExitStack() as ctx:
            _emit(nc, tc, ctx)
    nc.compile()
    _CACHED_NC = nc
    return nc


def _host_inputs(x, w_attn, b_attn, w_proj, b_proj):
    """Build the 8 per-core input maps."""
    import ml_dtypes
    mmnp = ml_dtypes.bfloat16 if MM_DT == BF16 else np.float32
    x = np.asarray(x, dtype=np.float32)
    w_attn = np.asarray(w_attn, dtype=np.float32)
    b_attn = np.asarray(b_attn, dtype=np.float32)
    w_proj = np.asarray(w_proj, dtype=np.float32)
    b_proj = np.asarray(b_proj, dtype=np.float32)

    scale = np.float32(1.0 / np.sqrt(D))
    # additive causal mask: -1e9 where query < key (strict lower triangle)
    mask = np.tril(np.ones((128, 128), dtype=np.float32), -1) * np.float32(-1e9)
    ident = np.eye(128, dtype=np.float32)

    xT_b = [np.ascontiguousarray(x[b].T).astype(mmnp) for b in range(B)]

    in_maps = []
    for core in range(N_CORES):
        b, hp = divmod(core, 4)
        r0 = 2 * hp * 64  # first row of this core's head-pair slice
        qr = w_attn[r0:r0 + 128] * scale
        kr = w_attn[C + r0:C + r0 + 128]
        vr = w_attn[2 * C + r0:2 * C + r0 + 128]
        wqkvT = np.ascontiguousarray(np.concatenate([qr, kr, vr], axis=0).T)
        bq = b_attn[r0:r0 + 128] * scale
        bk = b_attn[C + r0:C + r0 + 128]
        bv = b_attn[2 * C + r0:2 * C + r0 + 128]
        bqkv = np.ascontiguousarray(np.stack([bq, bk, bv], axis=1))
        wpT = np.ascontiguousarray(w_proj[:, r0:r0 + 128].T)
        in_maps.append({
            "xT": xT_b[b],
            "wqkvT": wqkvT.astype(mmnp),
            "bqkv": bqkv,
            "wpT": wpT.astype(mmnp),
            "mask01": mask.astype(mmnp),
            "ident": ident.astype(mmnp),
        })
    return in_maps


def _gather(results, b_proj):
    out = np.empty((B, T, C), dtype=np.float32)
    for b in range(B):
        acc = results[b * 4]["outT"].astype(np.float32).copy()
        for hp in range(1, 4):
            acc += results[b * 4 + hp]["outT"]
        out[b] = acc.T + np.asarray(b_proj, dtype=np.float32)
    return out


def kernel(x, w_attn, b_attn, w_proj, b_proj, _run_kwargs=None):
    nc = _build_program()
    in_maps = _host_inputs(x, w_attn, b_attn, w_proj, b_proj)
    kw = dict(_run_kwargs or {})
    res = bass_utils.run_bass_kernel_spmd(nc, in_maps,
                                          core_ids=list(range(N_CORES)), **kw)
    out = _gather(res.results, b_proj)
    if _run_kwargs is not None:
        kernel.last_result = res
    return out
